# revision 1
# baseline (speedup 1.0000x reference)
"""AttentionPairBias Trainium2 kernel (8-core SPMD, row-sharded).

Sharding: core c owns query rows i in [128c, 128c+128) and the matching z
rows z[:, i_shard, :, :]. k/v shards are computed from each core's own rows
and AllGathered. The pair-bias path computes LayerNorm stats with bn_stats
(two j's per op via the even/odd interleave), projects raw z16 through wz on
the PE (per-j transposes), and folds LN mean/rstd in as a post-matmul
correction:
    bias_h(i,j) = rs_ij * (P_raw_h(i,j) - mu_ij * c1_h)   [+ const_h, dropped:
softmax is shift-invariant per row]. z_norm_w is folded into wz, z_norm_b
drops with the constant. No softmax max-subtraction: logits are O(1) by
construction (|logit| < ~4), exact in fp32 exp.
"""
import numpy as np

import concourse.bass as bass
import concourse.tile as tile_mod
from concourse import mybir
from concourse.tile import TileContext
from concourse.masks import make_identity
from concourse.vector_clock import ScopedClock

F32 = mybir.dt.float32
F16 = mybir.dt.float16

S = 1024          # sequence length
DS = 1024         # model dim
H = 16            # heads
HD = 64           # head dim
DZ = 128          # pair dim
NCORES = 8
SI = S // NCORES  # 128 query rows per core


# ---------------------------------------------------------------------------
# Framework patch: this walrus build accepts only ONE semaphore wait per
# instruction, but TileContext's final drain aggregates every outstanding sem
# wait onto a single SP Drain. Split the waits across a chain of Drains.
# ---------------------------------------------------------------------------
def _patched_drain_and_barrier(self, tick_clock, wait_clock):
    nc = self.nc
    drain_inst = nc.sync.drain()
    wait_clock.add_sem_waits(
        drain_inst.ins, ScopedClock({None: tick_clock.global_clock})
    )
    si = drain_inst.ins.sync_info
    if si is not None and si.on_wait is not None and len(si.on_wait) > 1:
        extra = list(si.on_wait[1:])
        del si.on_wait[1:]
        for w in extra:
            d2 = nc.sync.drain()
            si2 = d2.ins.sync_info
            if si2 is None:
                d2.ins.sync_info = mybir.SyncInfo(on_wait=[w], on_update=[])
            else:
                si2.on_wait.append(w)
    nc.all_engine_barrier()
    assert self.sems is not None
    popped = nc._tile_sem_poison_stack.pop()
    assert popped is self._sem_poison
    nc.clear_and_free_semaphores(list(self.sems.allocated().values()))
    nc.all_engine_barrier()


def _install_patches():
    tile_mod.TileContext._drain_and_barrier = _patched_drain_and_barrier


_install_patches()


def _split_multiwait(nc):
    """This walrus build accepts at most one semaphore wait per instruction;
    Tile emits more when an op depends on producers on several engines. Hoist
    all-but-one wait onto same-engine NOPs inserted just before. (HW/walrus
    only — CoreSim can't run the unregistered NOPs.)"""
    for fn in nc.m.functions:
        for bb in fn.blocks:
            out = []
            changed = False
            for inst in bb.instructions:
                si = inst.sync_info
                if si is not None and si.on_wait is not None and len(si.on_wait) > 1:
                    extra = list(si.on_wait[:-1])
                    del si.on_wait[:-1]
                    for w in extra:
                        out.append(mybir.InstNoOp(
                            name=nc.get_next_instruction_name(),
                            engine=inst.engine,
                            bass_nofuse=True,
                            sync_info=mybir.SyncInfo(on_wait=[w], on_update=[]),
                        ))
                    changed = True
                out.append(inst)
            if changed:
                bb.instructions[:] = out


def _bn_stats_noopt(nc, out, in_):
    """bn_stats with opt=False AP lowering (keeps the interleaved view)."""
    return nc.vector.add_instruction(
        mybir.InstBNStats(
            name=nc.get_next_instruction_name(),
            ins=[nc.vector.lower_ap(in_, opt=False)],
            outs=[nc.vector.lower_ap(out, opt=False)],
        )
    )


def _bcast(ap, dims, extra_offset=0):
    return bass.AP(tensor=ap.tensor, offset=ap.offset + extra_offset, ap=dims)


def build_nc(split_waits=True, interleave_stats=True):
    nc = bass.Bass("TRN2", target_bir_lowering=False, debug=False,
                   num_devices=NCORES)

    z_sh = nc.dram_tensor("z_sh", [SI, S, DZ], F32, kind="ExternalInput").ap()
    sTi16 = nc.dram_tensor("sTi16", [DS, SI], F16, kind="ExternalInput").ap()
    wqT16 = nc.dram_tensor("wqT16", [DS, DS], F16, kind="ExternalInput").ap()
    wkT16 = nc.dram_tensor("wkT16", [DS, DS], F16, kind="ExternalInput").ap()
    wvT16 = nc.dram_tensor("wvT16", [DS, DS], F16, kind="ExternalInput").ap()
    wgT16 = nc.dram_tensor("wgT16", [DS, DS], F16, kind="ExternalInput").ap()
    woT16 = nc.dram_tensor("woT16", [DS, DS], F16, kind="ExternalInput").ap()
    wz16 = nc.dram_tensor("wz16", [DZ, H], F16, kind="ExternalInput").ap()
    c1h = nc.dram_tensor("c1h", [1, H], F32, kind="ExternalInput").ap()
    bq8 = nc.dram_tensor("bq8", [DS, 1], F32, kind="ExternalInput").ap()
    out_sh = nc.dram_tensor("out_sh", [SI, DS], F32, kind="ExternalOutput").ap()

    kv_agi = nc.dram_tensor("kv_agi", [SI, 2 * DS], F16)
    kv_ago = nc.dram_tensor("kv_ago", [S, 2 * DS], F16, addr_space="Shared")

    with TileContext(nc, pool_alloc_mode="queue") as tc:
        _emit(nc, tc, z_sh, sTi16, wqT16, wkT16, wvT16, wgT16, woT16,
              wz16, c1h, bq8, out_sh, kv_agi, kv_ago, interleave_stats)
    if split_waits:
        _split_multiwait(nc)
    return nc


def _emit(nc, tc, z_sh, sTi16, wqT16, wkT16, wvT16, wgT16, woT16,
          wz16, c1h, bq8, out_sh, kv_agi, kv_ago, interleave_stats):
    from contextlib import ExitStack
    AL = mybir.AluOpType
    AF = mybir.ActivationFunctionType

    KT = 8   # 1024/128 K tiles
    G = 8    # j-group size in the z pipeline
    NG = S // G          # 128 groups
    JB = 32              # j's per P psum bank
    NB = S // JB         # 32 P banks
    RND = 256            # j's per stats-finalize round
    NR = S // RND        # 4 rounds

    ctx = ExitStack()
    with ctx:
        consts = ctx.enter_context(tc.tile_pool(name="consts", bufs=1))
        persist = ctx.enter_context(tc.tile_pool(name="persist", bufs=1))

        ident16 = consts.tile([128, 128], F16)
        make_identity(nc, ident16)
        wz_sb = consts.tile([DZ, H], F16)
        nc.sync.dma_start(out=wz_sb, in_=wz16)
        c1h_sb = consts.tile([128, H], F32)
        nc.sync.dma_start(out=c1h_sb, in_=_bcast(c1h, [[0, 128], [1, H]]))
        bq_sb = consts.tile([128, KT], F32)
        nc.sync.dma_start(out=bq_sb, in_=bq8.rearrange("(m p) o -> p (m o)", p=128))
        eps_sb = consts.tile([128, 1], F32)
        nc.vector.memset(eps_sb, 1e-5)

        # persistent SBUF tensors
        kT_sb = persist.tile([128, KT, S], F16)     # [d-part, d-tile, j]
        v_sb = persist.tile([128, KT, DS], F16)     # [j-part, j-tile, d]
        qT_sb = persist.tile([128, KT, SI], F16)    # [d-part, d-tile, i]
        g16 = persist.tile([128, DS], F16)          # [i, d]
        st_sb = persist.tile([128, S // 2, 6], F32)  # bn_stats (j-pair, 6)
        rs = persist.tile([128, S], F32)            # 1/sqrt(var+eps)
        murs = persist.tile([128, S], F32)          # mu*rs
        sums = persist.tile([128, H], F32)
        inv = persist.tile([128, H], F32)
        og16 = persist.tile([128, DS], F16)
        ogT_sb = persist.tile([128, KT, SI], F16)
        out_sb = persist.tile([128, DS], F32)

        # ---------------- Phase A: projections + kv AllGather ----------------
        with (
            tc.tile_pool(name="wpool", bufs=1) as wpool,
            tc.tile_pool(name="apsum", bufs=2, space="PSUM") as apsum,
        ):
            sTi_sb = wpool.tile([128, KT, SI], F16)
            nc.sync.dma_start(
                out=sTi_sb, in_=sTi16.rearrange("(m p) n -> p m n", p=128))
            wq_sb = wpool.tile([128, KT, DS], F16)
            nc.sync.dma_start(
                out=wq_sb, in_=wqT16.rearrange("(m p) n -> p m n", p=128))
            wk_sb = wpool.tile([128, KT, DS], F16)
            nc.sync.dma_start(
                out=wk_sb, in_=wkT16.rearrange("(m p) n -> p m n", p=128))
            wv_sb = wpool.tile([128, KT, DS], F16)
            nc.sync.dma_start(
                out=wv_sb, in_=wvT16.rearrange("(m p) n -> p m n", p=128))
            wg_sb = wpool.tile([128, KT, DS], F16)
            nc.sync.dma_start(
                out=wg_sb, in_=wgT16.rearrange("(m p) n -> p m n", p=128))

            # k/v shards for own rows: [128 j, 1024 d], then AllGather
            kv_sh = wpool.tile([128, 2, DS], F16)
            for which, w_sb in ((0, wk_sb), (1, wv_sb)):
                for n in range(2):
                    kvp = apsum.tile([128, 512], F32, tag="kvp")
                    for k in range(KT):
                        nc.tensor.matmul(kvp, sTi_sb[:, k, :],
                                         w_sb[:, k, 512 * n:512 * (n + 1)],
                                         start=(k == 0), stop=(k == KT - 1))
                    nc.any.tensor_copy(kv_sh[:, which, 512 * n:512 * (n + 1)], kvp)
            nc.sync.dma_start(
                out=kv_agi.ap().rearrange("p (w n) -> p w n", w=2), in_=kv_sh)

            # qT[d, i] += bq  (wq, bq pre-scaled by 1/8 on host)
            for m in range(KT):
                qp = apsum.tile([128, SI], F32, tag="qp")
                for k in range(KT):
                    nc.tensor.matmul(qp, wq_sb[:, k, 128 * m:128 * (m + 1)],
                                     sTi_sb[:, k, :],
                                     start=(k == 0), stop=(k == KT - 1))
                nc.vector.tensor_scalar(
                    out=qT_sb[:, m, :], in0=qp, scalar1=bq_sb[:, m:m + 1],
                    scalar2=None, op0=AL.add)

            # g = sigmoid(s_i @ wg^T)   [i, d]
            for n in range(2):
                gp = apsum.tile([128, 512], F32, tag="gp")
                for k in range(KT):
                    nc.tensor.matmul(gp, sTi_sb[:, k, :],
                                     wg_sb[:, k, 512 * n:512 * (n + 1)],
                                     start=(k == 0), stop=(k == KT - 1))
                nc.scalar.activation(g16[:, 512 * n:512 * (n + 1)], gp,
                                     AF.Sigmoid)

        # P/bias buffers live B..C; allocated after phase A frees the weights
        biasp = ctx.enter_context(tc.tile_pool(name="biasp", bufs=1))
        P16 = biasp.tile([128, S, H], F16)          # P_raw [i, j, h]
        bias32 = biasp.tile([128, S, H], F32)       # corrected bias

        # ---------------- Phase B: z pipeline ----------------
        with (
            tc.tile_pool(name="zpool", bufs=8) as zpool,
            tc.tile_pool(name="ztpool", bufs=3) as ztpool,
            tc.tile_pool(name="zpsum", bufs=3, space="PSUM") as zpsum,
            tc.tile_pool(name="ppsum", bufs=2, space="PSUM") as ppsum,
            tc.tile_pool(name="stmp", bufs=2) as stmp,
        ):
            def finalize_round(r):
                # stats finalize (per parity; bn_stats cols: even j ->
                # [count, mean, M2] = 0..2, odd j -> 3..5)
                pr = slice(RND * r // 2, RND * (r + 1) // 2)   # pair indices
                for par in range(2):
                    stm = st_sb[:, pr, 1 + 3 * par:2 + 3 * par]
                    st2 = st_sb[:, pr, 2 + 3 * par:3 + 3 * par]
                    # strided output views over j (stride 2)
                    ro = _bcast(rs, [list(rs.ap[0]), [2, RND // 2], [0, 1]],
                                extra_offset=RND * r + par)
                    mo = _bcast(murs, [list(murs.ap[0]), [2, RND // 2], [0, 1]],
                                extra_offset=RND * r + par)
                    veps = stmp.tile([128, RND // 2, 1], F32, tag="veps")
                    nc.vector.tensor_scalar_mul(veps, st2, 1.0 / DZ)
                    sq = stmp.tile([128, RND // 2, 1], F32, tag="sq")
                    nc.scalar.activation(sq, veps, AF.Sqrt, bias=eps_sb)
                    nc.vector.reciprocal(ro, sq)
                    nc.vector.tensor_tensor(out=mo, in0=stm, in1=ro, op=AL.mult)

            def correct_bank(b):
                # bias = rs*P_raw - (mu*rs) x c1
                jb = slice(JB * b, JB * (b + 1))
                rs_rep = _bcast(rs, [list(rs.ap[0]), [1, JB], [0, H]],
                                extra_offset=JB * b)
                murs_rep = _bcast(murs, [list(murs.ap[0]), [1, JB], [0, H]],
                                  extra_offset=JB * b)
                c1_rep = _bcast(c1h_sb, [list(c1h_sb.ap[0]), [0, JB], [1, H]])
                t1 = stmp.tile([128, JB, H], F32, tag="t1")
                nc.vector.tensor_tensor(out=t1, in0=P16[:, jb, :], in1=rs_rep,
                                        op=AL.mult)
                t2 = stmp.tile([128, JB, H], F32, tag="t2")
                nc.gpsimd.tensor_tensor(out=t2, in0=murs_rep, in1=c1_rep,
                                        op=AL.mult)
                nc.gpsimd.tensor_tensor(out=bias32[:, jb, :], in0=t1, in1=t2,
                                        op=AL.subtract)

            pbank = None
            for jg in range(NG):
                j0 = jg * G
                z16 = zpool.tile([128, G, DZ], F16, tag="z16")
                nc.gpsimd.dma_start(out=z16, in_=z_sh[:, j0:j0 + G, :])

                # LayerNorm stats. Interleaved: one bn_stats per j-PAIR with
                # an even/odd interleave view [z-step 1 x 128, j-step 128 x 2]
                # -> even stats = first j, odd stats = second j.
                for t in range(G // 2):
                    iv = _bcast(z16, [list(z16.ap[0]), [1, DZ], [DZ, 2]],
                                extra_offset=2 * t * DZ)
                    _bn_stats_noopt(nc, st_sb[:, j0 // 2 + t, :], iv)

                # transpose each [128i, 128z] -> [128z, 128i] (f16, one bank)
                ztb = zpsum.tile([128, G, 128], F16, tag="ztb")
                for t in range(G):
                    nc.tensor.transpose(ztb[:, t, :], z16[:, t, :], ident16)
                zt_sb = ztpool.tile([128, G, 128], F16, tag="zt")
                nc.any.tensor_copy(zt_sb, ztb)

                # P_raw[i, h] per j, packed 32 j per psum bank
                if jg % 4 == 0:
                    pbank = ppsum.tile([128, JB, H], F32, tag="pbank")
                for t in range(G):
                    jj = (jg % 4) * G + t
                    nc.tensor.matmul(pbank[:, jj, :], zt_sb[:, t, :], wz_sb,
                                     start=True, stop=True)
                if jg % 4 == 3:
                    b = jg // 4
                    nc.any.tensor_copy(
                        P16[:, JB * b:JB * (b + 1), :], pbank)

                if jg == 12:
                    # collective on the gpsimd queue, emitted mid-loop so the
                    # z-load pipeline is already buffered ahead of the stall
                    nc.gpsimd.collective_compute(
                        "AllGather", AL.bypass, ins=[kv_agi.ap()],
                        outs=[kv_ago.ap()],
                        replica_groups=[list(range(NCORES))])
                if jg == 28:
                    # unpack the gathered k/v; build kT via PE transposes
                    kv_view = kv_ago.ap().rearrange(
                        "(t p) (w n) -> p t w n", p=128, w=2)
                    nc.sync.dma_start(out=v_sb, in_=kv_view[:, :, 1, :])
                    for m in range(KT):
                        knm = stmp.tile([128, KT, 128], F16, tag="knm")
                        nc.sync.dma_start(
                            out=knm, in_=kv_view[:, :, 0, 128 * m:128 * (m + 1)])
                        ktp = zpsum.tile([128, KT, 128], F16, tag="ktp")
                        for t in range(KT):
                            nc.tensor.transpose(ktp[:, t, :], knm[:, t, :],
                                                ident16)
                        nc.any.tensor_copy(
                            kT_sb[:, m, :].rearrange("p (t n) -> p t n", n=128),
                            ktp)
                # pipeline the finalize + corrections: after the last group
                # of round r, finalize its stats and correct its 8 banks.
                if (jg + 1) % (RND // G) == 0:
                    r = (jg + 1) // (RND // G) - 1
                    finalize_round(r)
                    for b in range(r * (RND // JB), (r + 1) * (RND // JB)):
                        correct_bank(b)

        # ---------------- Phase C: attention ----------------
        with (
            tc.tile_pool(name="scps", bufs=2, space="PSUM") as scps,
            tc.tile_pool(name="atps", bufs=2, space="PSUM") as atps,
            tc.tile_pool(name="ops", bufs=1, space="PSUM") as ops,
            tc.tile_pool(name="attn", bufs=2) as attnp,
        ):
            ob = ops.tile([128, 2, 8, HD], F32)
            for h in range(H):
                m, p0 = h // 2, 64 * (h % 2)
                scp = scps.tile([128, 2, 512], F32, tag="scp")
                for n in range(2):
                    nc.tensor.matmul(scp[:, n, :],
                                     qT_sb[p0:p0 + 64, m, :],
                                     kT_sb[p0:p0 + 64, m, 512 * n:512 * (n + 1)],
                                     start=True, stop=True)
                sc_sb = attnp.tile([128, S], F32, tag="sc")
                nc.vector.tensor_tensor(
                    out=sc_sb, in0=scp.rearrange("p a b -> p (a b)"),
                    in1=bias32[:, :, h], op=AL.add)
                attn16 = attnp.tile([128, S], F16, tag="at")
                nc.scalar.activation(attn16, sc_sb, AF.Exp)
                nc.vector.tensor_reduce(
                    out=sums[:, h:h + 1], in_=attn16, axis=mybir.AxisListType.X,
                    op=AL.add)
                atb = atps.tile([128, G, 128], F16, tag="atb")
                for t in range(G):
                    nc.tensor.transpose(atb[:, t, :],
                                        attn16[:, 128 * t:128 * (t + 1)],
                                        ident16)
                attnT = attnp.tile([128, G, 128], F16, tag="atT")
                nc.any.tensor_copy(attnT, atb)
                for t in range(G):
                    nc.tensor.matmul(ob[:, h // 8, h % 8, :], attnT[:, t, :],
                                     v_sb[:, t, HD * h:HD * (h + 1)],
                                     start=(t == 0), stop=(t == G - 1))
                if h % 8 == 7:
                    hb = h // 8
                    nc.vector.reciprocal(inv[:, 8 * hb:8 * (hb + 1)],
                                         sums[:, 8 * hb:8 * (hb + 1)])
                    for hh in range(8 * hb, 8 * (hb + 1)):
                        nc.vector.scalar_tensor_tensor(
                            out=og16[:, HD * hh:HD * (hh + 1)],
                            in0=ob[:, hb, hh % 8, :],
                            scalar=inv[:, hh:hh + 1],
                            in1=g16[:, HD * hh:HD * (hh + 1)],
                            op0=AL.mult, op1=AL.mult)

        # ---------------- Phase D: output projection ----------------
        with (
            tc.tile_pool(name="wopool", bufs=1) as wopool,
            tc.tile_pool(name="dpsum", bufs=2, space="PSUM") as dpsum,
        ):
            wo_sb = wopool.tile([128, KT, DS], F16)
            nc.sync.dma_start(
                out=wo_sb, in_=woT16.rearrange("(m p) n -> p m n", p=128))
            ogb = dpsum.tile([128, G, 128], F16, tag="ogb")
            for t in range(G):
                nc.tensor.transpose(ogb[:, t, :],
                                    og16[:, 128 * t:128 * (t + 1)], ident16)
            nc.any.tensor_copy(ogT_sb.rearrange("p k n -> p (k n)"),
                               ogb.rearrange("p k n -> p (k n)"))
            for n in range(2):
                op_ = dpsum.tile([128, 512], F32, tag="op")
                for k in range(KT):
                    nc.tensor.matmul(op_, ogT_sb[:, k, :],
                                     wo_sb[:, k, 512 * n:512 * (n + 1)],
                                     start=(k == 0), stop=(k == KT - 1))
                nc.any.tensor_copy(out_sb[:, 512 * n:512 * (n + 1)], op_)
            nc.sync.dma_start(out=out_sh, in_=out_sb)


def prep_inputs(s, z, wq, bq, wk, wv, wg, z_norm_w, z_norm_b, wz, wo):
    """Host-side prep: shard + transpose/cast weights. Returns in_maps."""
    s2 = np.asarray(s)[0]                     # [S, DS]
    sT = np.ascontiguousarray(s2.T).astype(np.float16)
    wqT = np.ascontiguousarray((np.asarray(wq) / 8.0).T).astype(np.float16)
    wkT = np.ascontiguousarray(np.asarray(wk).T).astype(np.float16)
    wvT = np.ascontiguousarray(np.asarray(wv).T).astype(np.float16)
    wgT = np.ascontiguousarray(np.asarray(wg).T).astype(np.float16)
    woT = np.ascontiguousarray(np.asarray(wo).T).astype(np.float16)
    wz_f = (np.asarray(z_norm_w)[:, None] * np.asarray(wz).T)  # [DZ, H]
    wz16 = wz_f.astype(np.float16)
    # c1_h = sum_z wz16[z, h] (f16-quantized wz to match the device P matmul)
    c1h = wz16.astype(np.float32).sum(axis=0)[None, :].astype(np.float32)
    bq8 = (np.asarray(bq) / 8.0).astype(np.float32)[:, None]
    z0 = np.asarray(z)[0]                     # [S, S, DZ]

    in_maps = []
    for c in range(NCORES):
        i0 = SI * c
        in_maps.append({
            "z_sh": np.ascontiguousarray(z0[i0:i0 + SI]).astype(np.float32),
            "sTi16": np.ascontiguousarray(sT[:, i0:i0 + SI]),
            "wqT16": wqT, "wkT16": wkT, "wvT16": wvT, "wgT16": wgT,
            "woT16": woT, "wz16": wz16, "c1h": c1h, "bq8": bq8,
        })
    return in_maps


_NC_CACHE = None


def _get_nc():
    global _NC_CACHE
    if _NC_CACHE is None:
        _NC_CACHE = build_nc()
    return _NC_CACHE


def kernel(**inputs):
    from concourse.bass_utils import run_bass_kernel_spmd
    nc = _get_nc()
    in_maps = prep_inputs(**inputs)
    res = run_bass_kernel_spmd(nc, in_maps, core_ids=list(range(NCORES)))
    out = np.empty((1, S, DS), dtype=np.float32)
    for c in range(NCORES):
        out[0, SI * c:SI * (c + 1), :] = res.results[c]["out_sh"]
    return out



# revision 6
# speedup vs baseline: 1.7886x; 1.7886x over previous
"""AttentionPairBias Trainium2 kernel (8-core SPMD, row-sharded).

Sharding: core c owns query rows i in [128c, 128c+128). The host ships its z
shard pre-transposed to [z, j, i] in f16, so the pair-bias projection is a
straight per-j matmul P(i,h) = zt_j^T @ [wz' | ones] with no on-device
transposes. LayerNorm folding:
  - z_norm_w folds into wz (host).
  - z_norm_b adds a j-independent constant per head -> dropped (softmax
    shift invariance).
  - the -mu correction folds into column-centered weights:
        sum_z (z - mu) wz = sum_z z (wz - mean_z wz) = sum_z z wz'
    so bias = rs * P' with P' = z @ wz'. Only rs = 1/sqrt(var+eps) remains,
    computed from S1 = sum_z z (ones column) and S2 = sum_z z^2 (DVE f16
    square + one-column matmul into the same psum bank).
k/v: kT is computed directly as wk @ s^T per core (no post-gather
transposes), packed with v into one AllGather. No softmax max-subtraction:
logits are O(1) by construction, exact in fp32 exp.
"""
import numpy as np

import concourse.bass as bass
import concourse.tile as tile_mod
from concourse import mybir
from concourse.tile import TileContext
from concourse.masks import make_identity
from concourse.vector_clock import ScopedClock

F32 = mybir.dt.float32
F16 = mybir.dt.float16

S = 1024          # sequence length
DS = 1024         # model dim
H = 16            # heads
HD = 64           # head dim
DZ = 128          # pair dim
NCORES = 8
SI = S // NCORES  # 128 query rows per core
KT = 8            # 1024/128 contraction tiles
JC = 32           # j's per z DMA chunk
NCH = S // JC     # 32 chunks
JB = 16           # j's per P psum bank (16*18 = 288 fp32 <= 512)


# ---------------------------------------------------------------------------
# Framework patch: this walrus build accepts only ONE semaphore wait per
# instruction, but TileContext's final drain aggregates every outstanding sem
# wait onto a single SP Drain. Split the waits across a chain of Drains.
# ---------------------------------------------------------------------------
def _patched_drain_and_barrier(self, tick_clock, wait_clock):
    nc = self.nc
    drain_inst = nc.sync.drain()
    wait_clock.add_sem_waits(
        drain_inst.ins, ScopedClock({None: tick_clock.global_clock})
    )
    si = drain_inst.ins.sync_info
    if si is not None and si.on_wait is not None and len(si.on_wait) > 1:
        extra = list(si.on_wait[1:])
        del si.on_wait[1:]
        for w in extra:
            d2 = nc.sync.drain()
            si2 = d2.ins.sync_info
            if si2 is None:
                d2.ins.sync_info = mybir.SyncInfo(on_wait=[w], on_update=[])
            else:
                si2.on_wait.append(w)
    nc.all_engine_barrier()
    assert self.sems is not None
    popped = nc._tile_sem_poison_stack.pop()
    assert popped is self._sem_poison
    nc.clear_and_free_semaphores(list(self.sems.allocated().values()))
    nc.all_engine_barrier()


def _install_patches():
    tile_mod.TileContext._drain_and_barrier = _patched_drain_and_barrier


_install_patches()


def _split_multiwait(nc):
    """This walrus build accepts at most one semaphore wait per instruction;
    Tile emits more when an op depends on producers on several engines. Hoist
    all-but-one wait onto same-engine NOPs inserted just before. (HW/walrus
    only — CoreSim can't run the unregistered NOPs.)"""
    for fn in nc.m.functions:
        for bb in fn.blocks:
            out = []
            changed = False
            for inst in bb.instructions:
                si = inst.sync_info
                if si is not None and si.on_wait is not None and len(si.on_wait) > 1:
                    extra = list(si.on_wait[:-1])
                    del si.on_wait[:-1]
                    for w in extra:
                        out.append(mybir.InstNoOp(
                            name=nc.get_next_instruction_name(),
                            engine=inst.engine,
                            bass_nofuse=True,
                            sync_info=mybir.SyncInfo(on_wait=[w], on_update=[]),
                        ))
                    changed = True
                out.append(inst)
            if changed:
                bb.instructions[:] = out


def _bcast(ap, dims, extra_offset=0):
    return bass.AP(tensor=ap.tensor, offset=ap.offset + extra_offset, ap=dims)


def build_nc(split_waits=True):
    nc = bass.Bass("TRN2", target_bir_lowering=False, debug=False,
                   num_devices=NCORES)

    zT_sh = nc.dram_tensor("zT_sh", [DZ, S, SI], F16, kind="ExternalInput").ap()
    sTi16 = nc.dram_tensor("sTi16", [DS, SI], F16, kind="ExternalInput").ap()
    wqT16 = nc.dram_tensor("wqT16", [DS, DS], F16, kind="ExternalInput").ap()
    wkT16 = nc.dram_tensor("wkT16", [DS, DS], F16, kind="ExternalInput").ap()
    wvT16 = nc.dram_tensor("wvT16", [DS, DS], F16, kind="ExternalInput").ap()
    wgT16 = nc.dram_tensor("wgT16", [DS, DS], F16, kind="ExternalInput").ap()
    woT16 = nc.dram_tensor("woT16", [DS, DS], F16, kind="ExternalInput").ap()
    wzo16 = nc.dram_tensor("wzo16", [DZ, H + 1], F16, kind="ExternalInput").ap()
    bq8 = nc.dram_tensor("bq8", [DS, 1], F32, kind="ExternalInput").ap()
    out_sh = nc.dram_tensor("out_sh", [SI, DS], F32, kind="ExternalOutput").ap()

    # packed kT | v shard for one AllGather: [2, 1024, 128] f16
    kv_agi = nc.dram_tensor("kv_agi", [2, DS, SI], F16)
    kv_ago = nc.dram_tensor("kv_ago", [2 * NCORES, DS, SI], F16,
                            addr_space="Shared")

    with TileContext(nc, pool_alloc_mode="queue") as tc:
        _emit(nc, tc, zT_sh, sTi16, wqT16, wkT16, wvT16, wgT16, woT16,
              wzo16, bq8, out_sh, kv_agi, kv_ago)
    if split_waits:
        _split_multiwait(nc)
    return nc


def _emit(nc, tc, zT_sh, sTi16, wqT16, wkT16, wvT16, wgT16, woT16,
          wzo16, bq8, out_sh, kv_agi, kv_ago):
    from contextlib import ExitStack
    AL = mybir.AluOpType
    AF = mybir.ActivationFunctionType

    ctx = ExitStack()
    with ctx:
        consts = ctx.enter_context(tc.tile_pool(name="consts", bufs=1))
        persist = ctx.enter_context(tc.tile_pool(name="persist", bufs=1))

        ident16 = consts.tile([128, 128], F16)
        make_identity(nc, ident16)
        wzo_sb = consts.tile([DZ, H + 1], F16)   # [wz' | ones]
        nc.sync.dma_start(out=wzo_sb, in_=wzo16)
        one16 = consts.tile([DZ, 1], F16)
        nc.vector.memset(one16, 1.0)
        bq_sb = consts.tile([128, KT], F32)
        nc.sync.dma_start(out=bq_sb, in_=bq8.rearrange("(m p) o -> p (m o)", p=128))
        eps_sb = consts.tile([128, 1], F32)
        nc.vector.memset(eps_sb, 1e-5)

        # persistent SBUF tensors
        kT_sb = persist.tile([128, KT, S], F16)     # [d-part, d-tile, j]
        v_sb = persist.tile([128, KT, DS], F16)     # [j-part, j-tile, d]
        qT_sb = persist.tile([128, KT, SI], F16)    # [d-part, d-tile, i]
        g16 = persist.tile([128, DS], F16)          # [i, d]
        bias16 = persist.tile([128, S, H], F16)     # rs * P'  [i, j, h]
        rs16 = persist.tile([128, S], F16)          # 1/sqrt(var+eps)
        sums = persist.tile([128, H], F32)
        inv = persist.tile([128, H], F32)
        og16 = persist.tile([128, DS], F16)
        ogT_sb = persist.tile([128, KT, SI], F16)
        out_sb = persist.tile([128, DS], F32)

        # ---------------- Phase A: projections + kv AllGather ----------------
        with (
            tc.tile_pool(name="wpool", bufs=1) as wpool,
            tc.tile_pool(name="apsum", bufs=2, space="PSUM") as apsum,
        ):
            sTi_sb = wpool.tile([128, KT, SI], F16)
            nc.sync.dma_start(
                out=sTi_sb, in_=sTi16.rearrange("(m p) n -> p m n", p=128))
            wq_sb = wpool.tile([128, KT, DS], F16)
            nc.sync.dma_start(
                out=wq_sb, in_=wqT16.rearrange("(m p) n -> p m n", p=128))
            wk_sb = wpool.tile([128, KT, DS], F16)
            nc.sync.dma_start(
                out=wk_sb, in_=wkT16.rearrange("(m p) n -> p m n", p=128))
            wv_sb = wpool.tile([128, KT, DS], F16)
            nc.scalar.dma_start(
                out=wv_sb, in_=wvT16.rearrange("(m p) n -> p m n", p=128))
            wg_sb = wpool.tile([128, KT, DS], F16)
            nc.scalar.dma_start(
                out=wg_sb, in_=wgT16.rearrange("(m p) n -> p m n", p=128))

            # kT shard [d, i_own] = wk @ s^T, straight to f16 staging
            kTi_sb = wpool.tile([128, KT, SI], F16)
            for m in range(KT):
                kp = apsum.tile([128, SI], F32, tag="kp")
                for k in range(KT):
                    nc.tensor.matmul(kp, wk_sb[:, k, 128 * m:128 * (m + 1)],
                                     sTi_sb[:, k, :],
                                     start=(k == 0), stop=(k == KT - 1))
                nc.scalar.activation(kTi_sb[:, m, :], kp, AF.Copy)
            nc.sync.dma_start(
                out=kv_agi.ap()[0].rearrange("(m p) i -> p m i", p=128),
                in_=kTi_sb)

            # v shard [j_own, d]
            vi_sb = wpool.tile([128, DS], F16)
            for n in range(2):
                vp = apsum.tile([128, 512], F32, tag="vp")
                for k in range(KT):
                    nc.tensor.matmul(vp, sTi_sb[:, k, :],
                                     wv_sb[:, k, 512 * n:512 * (n + 1)],
                                     start=(k == 0), stop=(k == KT - 1))
                nc.scalar.activation(vi_sb[:, 512 * n:512 * (n + 1)], vp, AF.Copy)
            nc.sync.dma_start(
                out=kv_agi.ap()[1].rearrange("(p a) b -> p (a b)", p=128),
                in_=vi_sb)

            # qT[d, i] += bq  (wq, bq pre-scaled by 1/8 on host)
            for m in range(KT):
                qp = apsum.tile([128, SI], F32, tag="qp")
                for k in range(KT):
                    nc.tensor.matmul(qp, wq_sb[:, k, 128 * m:128 * (m + 1)],
                                     sTi_sb[:, k, :],
                                     start=(k == 0), stop=(k == KT - 1))
                nc.vector.tensor_scalar(
                    out=qT_sb[:, m, :], in0=qp, scalar1=bq_sb[:, m:m + 1],
                    scalar2=None, op0=AL.add)

            # g = sigmoid(s_i @ wg^T)   [i, d]
            for n in range(2):
                gp = apsum.tile([128, 512], F32, tag="gp")
                for k in range(KT):
                    nc.tensor.matmul(gp, sTi_sb[:, k, :],
                                     wg_sb[:, k, 512 * n:512 * (n + 1)],
                                     start=(k == 0), stop=(k == KT - 1))
                nc.scalar.activation(g16[:, 512 * n:512 * (n + 1)], gp,
                                     AF.Sigmoid)

            # one packed collective, issued as soon as kv_agi is written
            nc.gpsimd.collective_compute(
                "AllGather", AL.bypass, ins=[kv_agi.ap()], outs=[kv_ago.ap()],
                replica_groups=[list(range(NCORES))])

        # ---------------- Phase B: z pipeline ----------------
        with (
            tc.tile_pool(name="ztpool", bufs=3) as ztpool,
            tc.tile_pool(name="sqpool", bufs=3) as sqpool,
            tc.tile_pool(name="ppsum", bufs=4, space="PSUM") as ppsum,
            tc.tile_pool(name="stmp", bufs=2) as stmp,
        ):
            def finalize_chunk(c, banks):
                # stats finalize + bias evacuation for chunk c (2 banks)
                j0 = c * JC
                s12 = stmp.tile([128, JC, 2], F32, tag="s12")
                for b, pbk in enumerate(banks):
                    nc.scalar.activation(s12[:, JB * b:JB * (b + 1), :],
                                         pbk[:, :, H:H + 2], AF.Copy)
                t1 = stmp.tile([128, JC], F32, tag="t1")
                nc.vector.tensor_tensor(out=t1, in0=s12[:, :, 0],
                                        in1=s12[:, :, 0], op=AL.mult)
                t2 = stmp.tile([128, JC], F32, tag="t2")
                nc.vector.scalar_tensor_tensor(
                    out=t2, in0=s12[:, :, 1], scalar=float(DZ), in1=t1,
                    op0=AL.mult, op1=AL.subtract)
                sqv = stmp.tile([128, JC], F32, tag="sqv")
                nc.scalar.activation(sqv, t2, AF.Sqrt, bias=eps_sb,
                                     scale=1.0 / (DZ * DZ))
                with nc.allow_low_precision("rs is O(1); f16 is plenty"):
                    nc.vector.reciprocal(rs16[:, j0:j0 + JC], sqv)
                # bias = rs * P'   (psum -> sbuf f16, fused scaling)
                for b, pbk in enumerate(banks):
                    jb0 = j0 + JB * b
                    rs_rep = _bcast(rs16, [list(rs16.ap[0]), [1, JB], [0, H]],
                                    extra_offset=jb0)
                    nc.vector.tensor_tensor(
                        out=bias16[:, jb0:jb0 + JB, :],
                        in0=pbk[:, :, 0:H], in1=rs_rep, op=AL.mult)

            pending = None
            for c in range(NCH):
                j0 = c * JC
                zt = ztpool.tile([128, JC, 128], F16, tag="zt")
                eng = nc.sync if c % 2 == 0 else nc.scalar
                eng.dma_start(out=zt, in_=zT_sh[:, j0:j0 + JC, :])

                ztsq = sqpool.tile([128, JC, 128], F16, tag="zq")
                if c % 3 == 2:
                    nc.scalar.activation(ztsq, zt, AF.Square)
                else:
                    nc.vector.tensor_tensor(out=ztsq, in0=zt, in1=zt,
                                            op=AL.mult)

                banks = []
                pb = None
                for t in range(JC):
                    jj = t % JB
                    if jj == 0:
                        pb = ppsum.tile([128, JB, H + 2], F32, tag="pb")
                        banks.append(pb)
                    nc.tensor.matmul(pb[:, jj, 0:H + 1], zt[:, t, :], wzo_sb,
                                     start=True, stop=True)
                    nc.tensor.matmul(pb[:, jj, H + 1:H + 2], ztsq[:, t, :],
                                     one16, start=True, stop=True)

                if pending is not None:
                    finalize_chunk(c - 1, pending)
                pending = banks
            finalize_chunk(NCH - 1, pending)

            # unpack the gathered kT / v (emitted after all z DMAs so the
            # sync queue never stalls the z pipeline behind the collective)
            kv_kT = kv_ago.ap().rearrange(
                "(c w) (m p) i -> w m p c i", c=NCORES, w=2, m=KT, p=128)
            for m in range(KT):
                nc.sync.dma_start(out=kT_sb[:, m, :], in_=kv_kT[0, m])
            kv_v = kv_ago.ap().rearrange(
                "(c w) (p a) b -> w c p a b", c=NCORES, w=2, p=128, a=KT)
            for c2 in range(NCORES):
                nc.sync.dma_start(out=v_sb[:, c2, :], in_=kv_v[1, c2])

        # ---------------- Phase C: attention ----------------
        with (
            tc.tile_pool(name="scps", bufs=2, space="PSUM") as scps,
            tc.tile_pool(name="atps", bufs=2, space="PSUM") as atps,
            tc.tile_pool(name="ops", bufs=1, space="PSUM") as ops,
            tc.tile_pool(name="attn", bufs=2) as attnp,
        ):
            ob = ops.tile([128, 2, 8, HD], F32)
            for h in range(H):
                m, p0 = h // 2, 64 * (h % 2)
                scp = scps.tile([128, 2, 512], F32, tag="scp")
                for n in range(2):
                    nc.tensor.matmul(scp[:, n, :],
                                     qT_sb[p0:p0 + 64, m, :],
                                     kT_sb[p0:p0 + 64, m, 512 * n:512 * (n + 1)],
                                     start=True, stop=True)
                sc_sb = attnp.tile([128, S], F32, tag="sc")
                nc.vector.tensor_tensor(
                    out=sc_sb, in0=scp.rearrange("p a b -> p (a b)"),
                    in1=bias16[:, :, h], op=AL.add)
                attn16 = attnp.tile([128, S], F16, tag="at")
                nc.scalar.activation(attn16, sc_sb, AF.Exp,
                                     accum_out=sums[:, h:h + 1])
                atb = atps.tile([128, 8, 128], F16, tag="atb")
                for t in range(8):
                    nc.tensor.transpose(atb[:, t, :],
                                        attn16[:, 128 * t:128 * (t + 1)],
                                        ident16)
                attnT = attnp.tile([128, 8, 128], F16, tag="atT")
                nc.scalar.activation(attnT, atb, AF.Copy)
                for t in range(8):
                    nc.tensor.matmul(ob[:, h // 8, h % 8, :], attnT[:, t, :],
                                     v_sb[:, t, HD * h:HD * (h + 1)],
                                     start=(t == 0), stop=(t == 7))
                if h % 8 == 7:
                    hb = h // 8
                    nc.vector.reciprocal(inv[:, 8 * hb:8 * (hb + 1)],
                                         sums[:, 8 * hb:8 * (hb + 1)])
                    for hh in range(8 * hb, 8 * (hb + 1)):
                        nc.vector.scalar_tensor_tensor(
                            out=og16[:, HD * hh:HD * (hh + 1)],
                            in0=ob[:, hb, hh % 8, :],
                            scalar=inv[:, hh:hh + 1],
                            in1=g16[:, HD * hh:HD * (hh + 1)],
                            op0=AL.mult, op1=AL.mult)

        # ---------------- Phase D: output projection ----------------
        with (
            tc.tile_pool(name="wopool", bufs=1) as wopool,
            tc.tile_pool(name="dpsum", bufs=2, space="PSUM") as dpsum,
        ):
            wo_sb = wopool.tile([128, KT, DS], F16)
            nc.sync.dma_start(
                out=wo_sb, in_=woT16.rearrange("(m p) n -> p m n", p=128))
            ogb = dpsum.tile([128, 8, 128], F16, tag="ogb")
            for t in range(8):
                nc.tensor.transpose(ogb[:, t, :],
                                    og16[:, 128 * t:128 * (t + 1)], ident16)
            nc.scalar.activation(ogT_sb.rearrange("p k n -> p (k n)"),
                                 ogb.rearrange("p k n -> p (k n)"), AF.Copy)
            for n in range(2):
                op_ = dpsum.tile([128, 512], F32, tag="op")
                for k in range(KT):
                    nc.tensor.matmul(op_, ogT_sb[:, k, :],
                                     wo_sb[:, k, 512 * n:512 * (n + 1)],
                                     start=(k == 0), stop=(k == KT - 1))
                nc.scalar.activation(out_sb[:, 512 * n:512 * (n + 1)], op_, AF.Copy)
            nc.sync.dma_start(out=out_sh, in_=out_sb)


def prep_inputs(s, z, wq, bq, wk, wv, wg, z_norm_w, z_norm_b, wz, wo):
    """Host-side prep: shard + transpose/cast. Returns in_maps."""
    s2 = np.asarray(s)[0]                     # [S, DS]
    sT = np.ascontiguousarray(s2.T).astype(np.float16)
    wqT = np.ascontiguousarray((np.asarray(wq) / 8.0).T).astype(np.float16)
    wkT = np.ascontiguousarray(np.asarray(wk).T).astype(np.float16)
    wvT = np.ascontiguousarray(np.asarray(wv).T).astype(np.float16)
    wgT = np.ascontiguousarray(np.asarray(wg).T).astype(np.float16)
    woT = np.ascontiguousarray(np.asarray(wo).T).astype(np.float16)
    # fold z_norm_w into wz, then column-center so the LN mean correction
    # vanishes: sum_z (z-mu) wz == sum_z z wz'
    wz_f = (np.asarray(z_norm_w)[:, None] * np.asarray(wz).T).astype(np.float64)
    wz_c = wz_f - wz_f.mean(axis=0, keepdims=True)
    wzo = np.concatenate(
        [wz_c, np.ones((DZ, 1), np.float64)], axis=1).astype(np.float16)
    bq8 = (np.asarray(bq) / 8.0).astype(np.float32)[:, None]
    z16 = np.asarray(z)[0].astype(np.float16)  # [S, S, DZ]

    in_maps = []
    for c in range(NCORES):
        i0 = SI * c
        zT = np.ascontiguousarray(z16[i0:i0 + SI].transpose(2, 1, 0))
        in_maps.append({
            "zT_sh": zT,
            "sTi16": np.ascontiguousarray(sT[:, i0:i0 + SI]),
            "wqT16": wqT, "wkT16": wkT, "wvT16": wvT, "wgT16": wgT,
            "woT16": woT, "wzo16": wzo, "bq8": bq8,
        })
    return in_maps


_NC_CACHE = None


def _get_nc():
    global _NC_CACHE
    if _NC_CACHE is None:
        _NC_CACHE = build_nc()
    return _NC_CACHE


def kernel(**inputs):
    from concourse.bass_utils import run_bass_kernel_spmd
    nc = _get_nc()
    in_maps = prep_inputs(**inputs)
    res = run_bass_kernel_spmd(nc, in_maps, core_ids=list(range(NCORES)))
    out = np.empty((1, S, DS), dtype=np.float32)
    for c in range(NCORES):
        out[0, SI * c:SI * (c + 1), :] = res.results[c]["out_sh"]
    return out
